# revision 33
# baseline (speedup 1.0000x reference)
"""Per-sample modulated conv2d (StyleGAN2-style Conv2dMod) on 8 trn2 NeuronCores,
computed via 1D Winograd F(2,3) along H (direct 3-tap conv along W).

Reference (fp32):
    scale[n,o] = (1+y[n,o]) * rsqrt(||W[o]||^2 * (1+y[n,o])^2 + 1e-8)
    out = conv2d(edge_pad(x), W) * scale[:, :, None, None]

F(2,3) along H: out row-pair [2ti, 2ti+1] = A^T [ (G w_h) .* (B^T d_h) ] where
d_h = 4 input rows 2ti..2ti+3 of the padded image.  1.5x fewer MACs than
direct: per core 192 matmuls of [128x128] @ [128x512] (98k PE cycles vs 147k).
2D Winograd (2.25x) was measured transform-bound: DVE/Pool run ~1.1/0.3
elem/lane/ns, so its 75k elem/lane of transforms exceed the PE win; the 1D
row transform is only ~17k elem/lane (4 ops of +-1 combos, contiguous rows).

Sharding (4x2): core (g, h) = samples {2g, 2g+1} x out-channels [256h, +256).
Matmul free dim 512 = 2 samples x 8 row-tiles x 32 cols.

Per-core pipeline (phase structure hides the input DMA under the stream):
  - paced DMA chain, 2 launch engines (sync/gpsimd), ~10 in flight (per-queue
    bandwidth needs >=3 concurrent transfers); per ic-chunk order [x, U]
  - 48 warm-up matmuls release the HAM clock gate during the initial DMA wait
  - row transform V_b = B^T-combos of stride-2 row views (vector engine, bf16)
  - phase tih0: all 8 psum groups (occ x b) accumulate icc-outer, so matmuls
    start as soon as the first ic-chunk lands (~5us after the queues open)
  - phase tih1: V and U resident -> clean ~218ns/matmul stream
  - output transform (A^T over b) with the demod scale folded in: the scalar
    engine evicts s*M0/s*M1 from PSUM, then vector scalar_tensor_tensor ops
    add +/-s*Mk (ops may read only one PSUM operand); odd rows last so only
    they gate the tail; bf16 out DMA split across both queues, fp32 on host
Host folds: U = G-transform of weights (bf16), edge-pad + layout x, +/-scale.
Whole-pipeline bf16 rel err ~5e-3 (validated vs fp32 reference).
"""

import os

import numpy as np

N, C_IN, H, W = 8, 512, 32, 32
C_OUT, K = 512, 3
EPS = 1e-08
HP, WP = H + 2, W + 2  # 34x34 edge-padded
SMP, OCC, ICC, B, KW = 2, 2, 4, 4, 3  # per-core: samples, oc/ic chunks, pts, taps
TI = 16  # row-tile count (32/2)
FREE = SMP * (TI // 2) * W  # 512 matmul free dim (half the tiles per phase)
NCORES = 8


def _build_bass():
    import concourse.bass as bass  # noqa: F401
    import concourse.mybir as mybir
    import concourse.tile as tile
    from concourse import bacc
    from concourse.tile_rust import add_dep_helper

    f32 = mybir.dt.float32
    bf16 = mybir.dt.bfloat16
    ADD = mybir.AluOpType.add
    SUB = mybir.AluOpType.subtract

    nc = bacc.Bacc("TRN2")

    # [ic%128, icc, smp, h, w] padded input
    xp_d = nc.dram_tensor("xp", [128, ICC, SMP, HP, WP], bf16, kind="ExternalInput")
    # [occ, icc, ic%128, b, kw, oc%128] H-transformed weights
    u_d = nc.dram_tensor("u", [OCC, ICC, 128, B, KW, 128], bf16, kind="ExternalInput")
    # [oc%128, occ, smp, +/-] demod scale and its negation
    sc_d = nc.dram_tensor("sc", [128, OCC, SMP, 2], f32, kind="ExternalInput")
    # [occ, oc%128, smp, h, w] scaled output
    out_d = nc.dram_tensor("out", [OCC, 128, SMP, H, W], bf16, kind="ExternalOutput")

    WARM_MMS = int(os.environ.get("WINO_WARM_MMS", "44"))
    CONC = int(os.environ.get("WINO_DMA_CONC", "10"))

    with tile.TileContext(nc) as tc:
        with (
            tc.tile_pool(name="singles", bufs=1) as singles,
            tc.tile_pool(name="psum", bufs=8, space="PSUM") as psum,
            tc.tile_pool(name="tap", bufs=2) as tap,
        ):
            sc_s = singles.tile([128, OCC, SMP, 2], f32, name="sc")
            nc.gpsimd.dma_start(out=sc_s, in_=sc_d[:])

            # PE warm-up during the initial DMA (HAM clock-gate release)
            if WARM_MMS:
                wdum = singles.tile([128, 128], bf16, name="wdum")
                nc.vector.memset(wdum, 0.0)
                warm_ps = psum.tile([128, FREE], f32, tag="ps", name="warm")
                for _ in range(WARM_MMS):
                    nc.tensor.matmul(
                        warm_ps[:32, :128], wdum[:, :32], wdum, start=True, stop=True
                    )

            # Paced DMA chain: at most CONC transfers in flight, arriving in
            # consumption order (per-icc: x pair, then that icc's U slices).
            dma_chain = []

            def chain_dma(out, in_):
                eng = (nc.sync, nc.gpsimd, nc.scalar)[len(dma_chain) % 3]
                bi = eng.dma_start(out=out, in_=in_)
                i = len(dma_chain)
                conc = 4 if i < 8 else CONC
                if i >= conc:
                    add_dep_helper(
                        bi.ins, dma_chain[i - conc].ins, sync=True, reason="dma pacing"
                    )
                dma_chain.append(bi)

            xp_t = singles.tile([128, ICC, SMP, HP, WP], bf16, name="xp")
            u_t = {
                occ: singles.tile([128, ICC, B, KW, 128], bf16, name=f"u{occ}")
                for occ in range(OCC)
            }
            HS = 18  # rows 0:18 cover everything phase tih0 reads
            for icc in range(ICC):
                for smp in range(SMP):
                    chain_dma(xp_t[:, icc, smp, :HS], xp_d[:, icc, smp, :HS])
                for occ in range(OCC):
                    chain_dma(u_t[occ][:, icc, : B // 2], u_d[occ, icc, :, : B // 2])
                    chain_dma(u_t[occ][:, icc, B // 2 :], u_d[occ, icc, :, B // 2 :])
            # x lower halves trail: phase tih1 only needs them ~20us later
            for icc in range(ICC):
                for smp in range(SMP):
                    chain_dma(xp_t[:, icc, smp, HS:], xp_d[:, icc, smp, HS:])

            # ---- row transform (vector engine, bf16) ----
            # V_b = B^T-row combos of stride-2 row views of the padded image:
            # b0 = R0-R2, b1 = R1+R2, b2 = R2-R1, b3 = R1-R3
            bt = [(0, 2, SUB), (1, 2, ADD), (2, 1, SUB), (1, 3, SUB)]
            v_t = singles.tile([128, B, ICC, SMP, TI, WP], bf16, name="vt")
            for rb in (0, 16):  # tile-halves; half 0 gates phase tih0
                for icc in range(ICC):
                    for b, (r0, r1, op) in enumerate(bt):
                        nc.vector.tensor_tensor(
                            v_t[:, b, icc, :, rb // 2 : rb // 2 + 8, :],
                            xp_t[:, icc, :, rb + r0 : rb + r0 + 15 : 2, :],
                            xp_t[:, icc, :, rb + r1 : rb + r1 + 15 : 2, :],
                            op,
                        )

            # ---- matmuls + output transform ----
            oimg2 = singles.tile([128, OCC, SMP, H, W], bf16, name="oimg2")

            def moving(b, icc, tih, kw):
                return v_t[:, b, icc, :, 8 * tih : 8 * tih + 8, kw : kw + W]

            def out_transform(occ, tih, ps):
                # even rows: s*((M0+M1)+M2) ; odd rows: s*((M1-M2)-M3), with the
                # demod scale s folded in: the scalar engine evicts s*M0 / s*M1,
                # then each vector op adds +/-s*Mk via its scalar operand.
                # Ops may read at most ONE PSUM operand; all are per-sample
                # halves (psum columns are [smp, ti, w]).
                r0 = 16 * tih
                POS, NEG = 0, 1

                def half(p, smp):
                    return p[:, 256 * smp : 256 * smp + 256]

                def sv(smp, sign):
                    return sc_s[:, occ, smp, sign : sign + 1]

                c0 = tap.tile([128, FREE], f32, tag="c0", name="c0")
                c1 = tap.tile([128, FREE], f32, tag="c1", name="c1")
                for smp in range(SMP):
                    nc.scalar.mul(half(c0, smp), half(ps[0], smp), sv(smp, POS))
                    nc.scalar.mul(half(c1, smp), half(ps[1], smp), sv(smp, POS))
                e1 = tap.tile([128, FREE], f32, tag="e1", name="e1")
                o1 = tap.tile([128, FREE], f32, tag="o1", name="o1")
                MUL = mybir.AluOpType.mult
                for smp in range(SMP):
                    nc.vector.scalar_tensor_tensor(
                        half(e1, smp), half(ps[1], smp), sv(smp, POS),
                        half(c0, smp), MUL, ADD,
                    )
                for smp in range(SMP):
                    nc.vector.scalar_tensor_tensor(
                        half(o1, smp), half(ps[2], smp), sv(smp, NEG),
                        half(c1, smp), MUL, ADD,
                    )
                for smp in range(SMP):
                    nc.vector.scalar_tensor_tensor(
                        oimg2[:, occ, smp, r0 : r0 + 16 : 2, :],
                        half(ps[2], smp), sv(smp, POS), half(e1, smp), MUL, ADD,
                    )
                for smp in range(SMP):
                    nc.vector.scalar_tensor_tensor(
                        oimg2[:, occ, smp, r0 + 1 : r0 + 16 : 2, :],
                        half(ps[3], smp), sv(smp, NEG), half(o1, smp), MUL, ADD,
                    )
                engs = (nc.scalar, nc.sync, nc.gpsimd)
                for i, (smp, rh) in enumerate(
                    (s, r) for s in range(SMP) for r in (0, 8)
                ):
                    ra = r0 + rh
                    engs[(2 * tih + i) % 3].dma_start(
                        out=out_d[occ, :, smp, ra : ra + 8, :],
                        in_=oimg2[:, occ, smp, ra : ra + 8, :],
                    )

            # phase tih0: 8 psum groups (occ x b) accumulate icc-outer so the
            # matmul stream is gated only on (x, U, V) of the current icc
            ps0 = {
                (occ, b): psum.tile([128, FREE], f32, tag="ps", name=f"p0{occ}{b}")
                for occ in range(OCC)
                for b in range(B)
            }
            for icc in range(ICC):
                for occ in range(OCC):
                    for b in range(B):
                        for kw in range(KW):
                            nc.tensor.matmul(
                                ps0[(occ, b)],
                                u_t[occ][:, icc, b, kw],
                                moving(b, icc, 0, kw),
                                start=(icc == 0 and kw == 0),
                                stop=(icc == ICC - 1 and kw == KW - 1),
                            )
            for occ in range(OCC):
                out_transform(occ, 0, [ps0[(occ, b)] for b in range(B)])

            # phase tih1: everything resident -> dense per-group accumulation
            for occ in range(OCC):
                ps1 = []
                for b in range(B):
                    p = psum.tile([128, FREE], f32, tag="ps", name=f"p1{occ}{b}")
                    for icc in range(ICC):
                        for kw in range(KW):
                            nc.tensor.matmul(
                                p,
                                u_t[occ][:, icc, b, kw],
                                moving(b, icc, 1, kw),
                                start=(icc == 0 and kw == 0),
                                stop=(icc == ICC - 1 and kw == KW - 1),
                            )
                    ps1.append(p)
                out_transform(occ, 1, ps1)

    nc.finalize()
    return nc


def _prep_host(x: np.ndarray, y: np.ndarray, weight: np.ndarray):
    """Returns per-core input maps (list of 8 dicts). All layout work in numpy."""
    import ml_dtypes

    s = y + 1.0  # [N, O]
    wsq = np.sum(weight * weight, axis=(1, 2, 3))  # [O]
    scale = s / np.sqrt(wsq[None, :] * (s * s) + EPS)  # [N, O]

    # U = G-transform of the 3 H-taps -> [b, O, I, kw]
    G = np.array([[1, 0, 0], [0.5, 0.5, 0.5], [0.5, -0.5, 0.5], [0, 0, 1]], np.float32)
    U = np.einsum("bk,oikl->boil", G, weight)  # [B, O, I, KW]

    u_h = []
    for h in range(2):
        Uh = U[:, 256 * h : 256 * h + 256, :, :]  # [B, 256, 512, KW]
        Uh = Uh.reshape(B, OCC, 128, ICC, 128, KW)  # [b, occ, oc_lo, icc, ic_p, kw]
        Uh = Uh.transpose(1, 3, 4, 0, 5, 2)  # [occ, icc, ic_p, b, kw, oc_lo]
        u_h.append(np.ascontiguousarray(Uh.astype(ml_dtypes.bfloat16)))

    xp = np.pad(x, ((0, 0), (0, 0), (1, 1), (1, 1)), mode="edge")  # [N, C, 34, 34]
    xp = xp.reshape(N, ICC, 128, HP, WP)  # [n, icc, ic_p, h, w]

    in_maps = []
    for core in range(NCORES):
        g, h = divmod(core, 2)
        xc = xp[2 * g : 2 * g + 2]  # [smp, icc, ic_p, h, w]
        xc = xc.transpose(2, 1, 0, 3, 4).reshape(128, ICC * SMP, HP, WP)
        scg = scale[2 * g : 2 * g + 2, 256 * h : 256 * h + 256]  # [smp, 256]
        scg = scg.reshape(SMP, OCC, 128).transpose(2, 1, 0)  # [oc_p, occ, smp]
        scg = np.stack([scg, -scg], axis=-1)  # [oc_p, occ, smp, +/-]
        in_maps.append(
            {
                "xp": np.ascontiguousarray(xc.astype(ml_dtypes.bfloat16)),
                "u": u_h[h],
                "sc": np.ascontiguousarray(scg.astype(np.float32)),
            }
        )
    return in_maps


def _gather(results) -> np.ndarray:
    out = np.empty((N, C_OUT, H, W), np.float32)
    for core in range(NCORES):
        g, h = divmod(core, 2)
        r = np.asarray(results[core]["out"]).astype(np.float32)  # [occ,128,smp,h,w]
        for occ in range(OCC):
            for smp in range(SMP):
                out[2 * g + smp, 256 * h + 128 * occ : 256 * h + 128 * (occ + 1)] = r[
                    occ, :, smp
                ]
    return out


def kernel(x: np.ndarray, y: np.ndarray, weight: np.ndarray) -> np.ndarray:
    from concourse.bass_utils import run_bass_kernel_spmd

    x = np.asarray(x, dtype=np.float32)
    y = np.asarray(y, dtype=np.float32)
    weight = np.asarray(weight, dtype=np.float32)

    in_maps = _prep_host(x, y, weight)
    nc = _build_bass()
    results = run_bass_kernel_spmd(nc, in_maps, core_ids=list(range(NCORES))).results
    return _gather(results)


# revision 34
# speedup vs baseline: 1.0152x; 1.0152x over previous
"""Per-sample modulated conv2d (StyleGAN2-style Conv2dMod) on 8 trn2 NeuronCores,
computed via 1D Winograd F(2,3) along H (direct 3-tap conv along W).

Reference (fp32):
    scale[n,o] = (1+y[n,o]) * rsqrt(||W[o]||^2 * (1+y[n,o])^2 + 1e-8)
    out = conv2d(edge_pad(x), W) * scale[:, :, None, None]

F(2,3) along H: out row-pair [2ti, 2ti+1] = A^T [ (G w_h) .* (B^T d_h) ] where
d_h = 4 input rows 2ti..2ti+3 of the padded image.  1.5x fewer MACs than
direct: per core 192 matmuls of [128x128] @ [128x512] (98k PE cycles vs 147k).
2D Winograd (2.25x) was measured transform-bound: DVE/Pool run ~1.1/0.3
elem/lane/ns, so its 75k elem/lane of transforms exceed the PE win; the 1D
row transform is only ~17k elem/lane (4 ops of +-1 combos, contiguous rows).

Sharding (4x2): core (g, h) = samples {2g, 2g+1} x out-channels [256h, +256).
Matmul free dim 512 = 2 samples x 8 row-tiles x 32 cols.

Per-core pipeline (phase structure hides the input DMA under the stream):
  - paced DMA chain, 2 launch engines (sync/gpsimd), ~10 in flight (per-queue
    bandwidth needs >=3 concurrent transfers); per ic-chunk order [x, U]
  - 48 warm-up matmuls release the HAM clock gate during the initial DMA wait
  - row transform V_b = B^T-combos of stride-2 row views (vector engine, bf16)
  - phase tih0: all 8 psum groups (occ x b) accumulate icc-outer, so matmuls
    start as soon as the first ic-chunk lands (~5us after the queues open)
  - phase tih1: V and U resident -> clean ~218ns/matmul stream
  - output transform (A^T over b) with the demod scale folded in: the scalar
    engine evicts s*M0/s*M1 from PSUM, then vector scalar_tensor_tensor ops
    add +/-s*Mk (ops may read only one PSUM operand); odd rows last so only
    they gate the tail; bf16 out DMA split across both queues, fp32 on host
Host folds: U = G-transform of weights (bf16), edge-pad + layout x, +/-scale.
Whole-pipeline bf16 rel err ~5e-3 (validated vs fp32 reference).
"""

import os

import numpy as np

N, C_IN, H, W = 8, 512, 32, 32
C_OUT, K = 512, 3
EPS = 1e-08
HP, WP = H + 2, W + 2  # 34x34 edge-padded
SMP, OCC, ICC, B, KW = 2, 2, 4, 4, 3  # per-core: samples, oc/ic chunks, pts, taps
TI = 16  # row-tile count (32/2)
FREE = SMP * (TI // 2) * W  # 512 matmul free dim (half the tiles per phase)
NCORES = 8


def _build_bass():
    import concourse.bass as bass  # noqa: F401
    import concourse.mybir as mybir
    import concourse.tile as tile
    from concourse import bacc
    from concourse.tile_rust import add_dep_helper

    f32 = mybir.dt.float32
    bf16 = mybir.dt.bfloat16
    ADD = mybir.AluOpType.add
    SUB = mybir.AluOpType.subtract

    nc = bacc.Bacc("TRN2")

    # [ic%128, icc, smp, h, w] padded input
    xp_d = nc.dram_tensor("xp", [128, ICC, SMP, HP, WP], bf16, kind="ExternalInput")
    # [occ, icc, ic%128, b, kw, oc%128] H-transformed weights
    u_d = nc.dram_tensor("u", [OCC, ICC, 128, B, KW, 128], bf16, kind="ExternalInput")
    # [oc%128, occ, smp, +/-] demod scale and its negation
    sc_d = nc.dram_tensor("sc", [128, OCC, SMP, 2], f32, kind="ExternalInput")
    # [occ, oc%128, smp, h, w] scaled output
    out_d = nc.dram_tensor("out", [OCC, 128, SMP, H, W], bf16, kind="ExternalOutput")

    WARM_MMS = int(os.environ.get("WINO_WARM_MMS", "48"))
    CONC = int(os.environ.get("WINO_DMA_CONC", "10"))

    with tile.TileContext(nc) as tc:
        with (
            tc.tile_pool(name="singles", bufs=1) as singles,
            tc.tile_pool(name="psum", bufs=8, space="PSUM") as psum,
            tc.tile_pool(name="tap", bufs=2) as tap,
        ):
            sc_s = singles.tile([128, OCC, SMP, 2], f32, name="sc")
            nc.gpsimd.dma_start(out=sc_s, in_=sc_d[:])

            # PE warm-up during the initial DMA (HAM clock-gate release)
            if WARM_MMS:
                wdum = singles.tile([128, 128], bf16, name="wdum")
                nc.vector.memset(wdum, 0.0)
                warm_ps = psum.tile([128, FREE], f32, tag="ps", name="warm")
                for _ in range(WARM_MMS):
                    nc.tensor.matmul(
                        warm_ps[:32, :128], wdum[:, :32], wdum, start=True, stop=True
                    )

            # Paced DMA chain: at most CONC transfers in flight, arriving in
            # consumption order (per-icc: x pair, then that icc's U slices).
            dma_chain = []

            def chain_dma(out, in_):
                eng = (nc.sync, nc.gpsimd, nc.scalar)[len(dma_chain) % 3]
                bi = eng.dma_start(out=out, in_=in_)
                i = len(dma_chain)
                conc = 4 if i < 8 else CONC
                if i >= conc:
                    add_dep_helper(
                        bi.ins, dma_chain[i - conc].ins, sync=True, reason="dma pacing"
                    )
                dma_chain.append(bi)

            xp_t = singles.tile([128, ICC, SMP, HP, WP], bf16, name="xp")
            u_t = {
                occ: singles.tile([128, ICC, B, KW, 128], bf16, name=f"u{occ}")
                for occ in range(OCC)
            }
            HS = 18  # rows 0:18 cover everything phase tih0 reads
            for icc in range(ICC):
                for smp in range(SMP):
                    chain_dma(xp_t[:, icc, smp, :HS], xp_d[:, icc, smp, :HS])
                for occ in range(OCC):
                    chain_dma(u_t[occ][:, icc, : B // 2], u_d[occ, icc, :, : B // 2])
                    chain_dma(u_t[occ][:, icc, B // 2 :], u_d[occ, icc, :, B // 2 :])
            # x lower halves trail: phase tih1 only needs them ~20us later
            for icc in range(ICC):
                for smp in range(SMP):
                    chain_dma(xp_t[:, icc, smp, HS:], xp_d[:, icc, smp, HS:])

            # ---- row transform (vector engine, bf16) ----
            # V_b = B^T-row combos of stride-2 row views of the padded image:
            # b0 = R0-R2, b1 = R1+R2, b2 = R2-R1, b3 = R1-R3
            bt = [(0, 2, SUB), (1, 2, ADD), (2, 1, SUB), (1, 3, SUB)]
            v_t = singles.tile([128, B, ICC, SMP, TI, WP], bf16, name="vt")
            for rb in (0, 16):  # tile-halves; half 0 gates phase tih0
                for icc in range(ICC):
                    for b, (r0, r1, op) in enumerate(bt):
                        nc.vector.tensor_tensor(
                            v_t[:, b, icc, :, rb // 2 : rb // 2 + 8, :],
                            xp_t[:, icc, :, rb + r0 : rb + r0 + 15 : 2, :],
                            xp_t[:, icc, :, rb + r1 : rb + r1 + 15 : 2, :],
                            op,
                        )

            # ---- matmuls + output transform ----
            oimg2 = singles.tile([128, OCC, SMP, H, W], bf16, name="oimg2")

            def moving(b, icc, tih, kw):
                return v_t[:, b, icc, :, 8 * tih : 8 * tih + 8, kw : kw + W]

            def out_transform(occ, tih, ps):
                # even rows: s*((M0+M1)+M2) ; odd rows: s*((M1-M2)-M3), with the
                # demod scale s folded in: the scalar engine evicts s*M0 / s*M1,
                # then each vector op adds +/-s*Mk via its scalar operand.
                # Ops may read at most ONE PSUM operand; all are per-sample
                # halves (psum columns are [smp, ti, w]).
                r0 = 16 * tih
                POS, NEG = 0, 1

                def half(p, smp):
                    return p[:, 256 * smp : 256 * smp + 256]

                def sv(smp, sign):
                    return sc_s[:, occ, smp, sign : sign + 1]

                c0 = tap.tile([128, FREE], f32, tag="c0", name="c0")
                c1 = tap.tile([128, FREE], f32, tag="c1", name="c1")
                for smp in range(SMP):
                    nc.scalar.mul(half(c0, smp), half(ps[0], smp), sv(smp, POS))
                    nc.scalar.mul(half(c1, smp), half(ps[1], smp), sv(smp, POS))
                e1 = tap.tile([128, FREE], f32, tag="e1", name="e1")
                o1 = tap.tile([128, FREE], f32, tag="o1", name="o1")
                MUL = mybir.AluOpType.mult
                for smp in range(SMP):
                    nc.vector.scalar_tensor_tensor(
                        half(e1, smp), half(ps[1], smp), sv(smp, POS),
                        half(c0, smp), MUL, ADD,
                    )
                for smp in range(SMP):
                    nc.vector.scalar_tensor_tensor(
                        half(o1, smp), half(ps[2], smp), sv(smp, NEG),
                        half(c1, smp), MUL, ADD,
                    )
                for smp in range(SMP):
                    nc.vector.scalar_tensor_tensor(
                        oimg2[:, occ, smp, r0 : r0 + 16 : 2, :],
                        half(ps[2], smp), sv(smp, POS), half(e1, smp), MUL, ADD,
                    )
                for smp in range(SMP):
                    nc.vector.scalar_tensor_tensor(
                        oimg2[:, occ, smp, r0 + 1 : r0 + 16 : 2, :],
                        half(ps[3], smp), sv(smp, NEG), half(o1, smp), MUL, ADD,
                    )
                engs = (nc.scalar, nc.sync, nc.gpsimd)
                for i, (smp, rh) in enumerate(
                    (s, r) for s in range(SMP) for r in (0, 8)
                ):
                    ra = r0 + rh
                    engs[(2 * tih + i) % 3].dma_start(
                        out=out_d[occ, :, smp, ra : ra + 8, :],
                        in_=oimg2[:, occ, smp, ra : ra + 8, :],
                    )

            # phase tih0: 8 psum groups (occ x b) accumulate icc-outer so the
            # matmul stream is gated only on (x, U, V) of the current icc
            ps0 = {
                (occ, b): psum.tile([128, FREE], f32, tag="ps", name=f"p0{occ}{b}")
                for occ in range(OCC)
                for b in range(B)
            }
            for icc in range(ICC):
                for occ in range(OCC):
                    for b in range(B):
                        for kw in range(KW):
                            nc.tensor.matmul(
                                ps0[(occ, b)],
                                u_t[occ][:, icc, b, kw],
                                moving(b, icc, 0, kw),
                                start=(icc == 0 and kw == 0),
                                stop=(icc == ICC - 1 and kw == KW - 1),
                            )
            for occ in range(OCC):
                out_transform(occ, 0, [ps0[(occ, b)] for b in range(B)])

            # phase tih1: everything resident -> dense per-group accumulation
            for occ in range(OCC):
                ps1 = []
                for b in range(B):
                    p = psum.tile([128, FREE], f32, tag="ps", name=f"p1{occ}{b}")
                    for icc in range(ICC):
                        for kw in range(KW):
                            nc.tensor.matmul(
                                p,
                                u_t[occ][:, icc, b, kw],
                                moving(b, icc, 1, kw),
                                start=(icc == 0 and kw == 0),
                                stop=(icc == ICC - 1 and kw == KW - 1),
                            )
                    ps1.append(p)
                out_transform(occ, 1, ps1)

    nc.finalize()
    return nc


def _prep_host(x: np.ndarray, y: np.ndarray, weight: np.ndarray):
    """Returns per-core input maps (list of 8 dicts). All layout work in numpy."""
    import ml_dtypes

    s = y + 1.0  # [N, O]
    wsq = np.sum(weight * weight, axis=(1, 2, 3))  # [O]
    scale = s / np.sqrt(wsq[None, :] * (s * s) + EPS)  # [N, O]

    # U = G-transform of the 3 H-taps -> [b, O, I, kw]
    G = np.array([[1, 0, 0], [0.5, 0.5, 0.5], [0.5, -0.5, 0.5], [0, 0, 1]], np.float32)
    U = np.einsum("bk,oikl->boil", G, weight)  # [B, O, I, KW]

    u_h = []
    for h in range(2):
        Uh = U[:, 256 * h : 256 * h + 256, :, :]  # [B, 256, 512, KW]
        Uh = Uh.reshape(B, OCC, 128, ICC, 128, KW)  # [b, occ, oc_lo, icc, ic_p, kw]
        Uh = Uh.transpose(1, 3, 4, 0, 5, 2)  # [occ, icc, ic_p, b, kw, oc_lo]
        u_h.append(np.ascontiguousarray(Uh.astype(ml_dtypes.bfloat16)))

    xp = np.pad(x, ((0, 0), (0, 0), (1, 1), (1, 1)), mode="edge")  # [N, C, 34, 34]
    xp = xp.reshape(N, ICC, 128, HP, WP)  # [n, icc, ic_p, h, w]

    in_maps = []
    for core in range(NCORES):
        g, h = divmod(core, 2)
        xc = xp[2 * g : 2 * g + 2]  # [smp, icc, ic_p, h, w]
        xc = xc.transpose(2, 1, 0, 3, 4).reshape(128, ICC * SMP, HP, WP)
        scg = scale[2 * g : 2 * g + 2, 256 * h : 256 * h + 256]  # [smp, 256]
        scg = scg.reshape(SMP, OCC, 128).transpose(2, 1, 0)  # [oc_p, occ, smp]
        scg = np.stack([scg, -scg], axis=-1)  # [oc_p, occ, smp, +/-]
        in_maps.append(
            {
                "xp": np.ascontiguousarray(xc.astype(ml_dtypes.bfloat16)),
                "u": u_h[h],
                "sc": np.ascontiguousarray(scg.astype(np.float32)),
            }
        )
    return in_maps


def _gather(results) -> np.ndarray:
    out = np.empty((N, C_OUT, H, W), np.float32)
    for core in range(NCORES):
        g, h = divmod(core, 2)
        r = np.asarray(results[core]["out"]).astype(np.float32)  # [occ,128,smp,h,w]
        for occ in range(OCC):
            for smp in range(SMP):
                out[2 * g + smp, 256 * h + 128 * occ : 256 * h + 128 * (occ + 1)] = r[
                    occ, :, smp
                ]
    return out


def kernel(x: np.ndarray, y: np.ndarray, weight: np.ndarray) -> np.ndarray:
    from concourse.bass_utils import run_bass_kernel_spmd

    x = np.asarray(x, dtype=np.float32)
    y = np.asarray(y, dtype=np.float32)
    weight = np.asarray(weight, dtype=np.float32)

    in_maps = _prep_host(x, y, weight)
    nc = _build_bass()
    results = run_bass_kernel_spmd(nc, in_maps, core_ids=list(range(NCORES))).results
    return _gather(results)
